# revision 16
# baseline (speedup 1.0000x reference)
"""Fused transformer block (B=4, N=1024, C=768, H=12, HID=3072) on 8 TRN2
NeuronCores.

Sharding: data-parallel over (batch, seq-half). Core c handles batch c//2,
sequence half c%2 -> 512 output rows (natural order). Attention keys are
packed on the host: only tokens with mask==1 (plus zero padding up to a
128-multiple KV) are shipped as a separate xk array, so the score matrix,
Exp, and AV shrink by the masked fraction. Padding keys are excluded via a
per-head ones-column in v (softmax denominator) that is zeroed for pads.

Precision: large GEMMs run fp8e4 (TRN e4m3, max 240). DoubleRow perf mode
is used only where the per-instruction contraction is a full 2x128 (qkv, v,
proj, fc1, AV key-pairs) - that is where real HW gets its 2x. Scores run
fp8 at 64-deep contraction (same rate as bf16). fc2 stays bf16: fp8 on the
MLP dominates the error budget, and fc2-bf16 keeps rel err at ~1.3e-2 vs
the 2e-2 gate. Activations are scaled into fp8 range by power-of-2 factors
(SH for LN outputs, SQ for q/k, SV for v, SO for o) with per-tensor
power-of-2 weight scales from the host; descales fold into existing
psum-drain ops and the Exp scale operand.

Engine balance: LN mean/var on DVE (bn_stats), rstd batched on Act as
exp(-0.5*ln(var+eps)) (shares the Exp activation table with softmax - only
2 table switches per iteration, for Gelu), LN apply on GpSimd (Pool), psum
drains and descale-copies on DVE, Exp/Gelu on Act, matmuls+transposes on
PE. PSUM accumulation is fp32 throughout.
"""

import numpy as np
import ml_dtypes

import concourse.bass as bass
import concourse.bacc as bacc
import concourse.mybir as mybir
import concourse.tile as tile
from concourse.bass_utils import run_bass_kernel_spmd
from concourse.masks import make_identity

P = 128
DIM = 768
HEADS = 12
HD = 64
HID = 3072
EPS = 1e-5
NT_O = 4  # token tiles for the core's own 512 rows
KC = DIM // P  # 6
KH = HID // P  # 24
N_CORES = 8

SH = 16.0  # fp8 scale for LN1/LN2 outputs
SO = 32.0  # fp8 scale for attention output o
SQ = 32.0  # fp8 scale for q/k rows
SV = 64.0  # fp8 scale for v (and the mask column, so it cancels)

bf16 = mybir.dt.bfloat16
f8 = mybir.dt.float8e4
f32 = mybir.dt.float32
ALU = mybir.AluOpType
ACT_F = mybir.ActivationFunctionType
DR = mybir.MatmulPerfMode.DoubleRow


def _build(flags, repeat=1):
    nc = bacc.Bacc(None)

    sc = flags["scales"]
    KT = flags["KT"]  # key tiles (padded kept keys / 128)
    KV = KT * P
    d_qk = 1.0 / (SH * sc["wqk"])
    d_v = 1.0 / (SH * sc["wv"])
    d_p = 1.0 / (SO * sc["wp"])
    d_f1 = 1.0 / (SH * sc["wf1"])

    xq_e = nc.declare_dram_parameter("xq", [512, DIM], f32, isOutput=False)
    xk_e = nc.declare_dram_parameter("xk", [KV, DIM], f32, isOutput=False)
    m01_e = nc.declare_dram_parameter("m01", [P, KT], f32, isOutput=False)
    wqk_e = nc.declare_dram_parameter("wqk", [DIM, 2 * DIM], f8, isOutput=False)
    wv_e = nc.declare_dram_parameter("wv", [DIM, DIM], f8, isOutput=False)
    wp_e = nc.declare_dram_parameter("wp", [DIM, DIM], f8, isOutput=False)
    wf1_e = nc.declare_dram_parameter("wf1", [DIM, HID], f8, isOutput=False)
    wf2_e = nc.declare_dram_parameter("wf2", [HID, DIM], bf16, isOutput=False)
    y_e = nc.declare_dram_parameter("y", [512, DIM], f32, isOutput=True)

    opt = {}
    if flags["ln1_gb"]:
        opt["ln1g"] = nc.declare_dram_parameter("ln1g", [DIM], f32, isOutput=False)
        opt["ln1b"] = nc.declare_dram_parameter("ln1b", [DIM], f32, isOutput=False)
    if flags["ln2_gb"]:
        opt["ln2g"] = nc.declare_dram_parameter("ln2g", [DIM], f32, isOutput=False)
        opt["ln2b"] = nc.declare_dram_parameter("ln2b", [DIM], f32, isOutput=False)
    if flags["bqk"]:
        opt["bqk"] = nc.declare_dram_parameter("bqk", [2 * DIM], f32, isOutput=False)
    if flags["bv"]:
        opt["bv"] = nc.declare_dram_parameter("bv", [DIM], f32, isOutput=False)
    if flags["bp"]:
        opt["bp"] = nc.declare_dram_parameter("bp", [DIM], f32, isOutput=False)
    if flags["bf1"]:
        opt["bf1"] = nc.declare_dram_parameter("bf1", [HID], f32, isOutput=False)
    if flags["bf2"]:
        opt["bf2"] = nc.declare_dram_parameter("bf2", [DIM], f32, isOutput=False)

    def bcast(ap):
        # replicate a [D] DRAM vector across all 128 partitions for DMA
        return bass.AP(tensor=ap.tensor, offset=ap.offset, ap=[[0, P], *ap.ap])

    with tile.TileContext(nc) as tc:
        import contextlib

        with contextlib.ExitStack() as ctx:
            singles = ctx.enter_context(tc.tile_pool(name="singles", bufs=1))
            lnp = ctx.enter_context(tc.tile_pool(name="ln", bufs=4))
            htmp = ctx.enter_context(tc.tile_pool(name="htmp", bufs=2))
            big = ctx.enter_context(tc.tile_pool(name="big", bufs=1))
            ppool = ctx.enter_context(tc.tile_pool(name="pT", bufs=2))
            tps = ctx.enter_context(tc.tile_pool(name="tps", bufs=1, space="PSUM"))
            mmps = ctx.enter_context(tc.tile_pool(name="mmps", bufs=3, space="PSUM"))
            sps = ctx.enter_context(tc.tile_pool(name="sps", bufs=2, space="PSUM"))

            # --- constants ---
            eps_t = singles.tile([P, 1], f32)
            nc.vector.memset(eps_t, EPS / (SH * SH))
            ident = singles.tile([P, P], bf16)
            make_identity(nc, ident)
            m01_sb = singles.tile([P, KT], f32)
            nc.sync.dma_start(out=m01_sb, in_=m01_e[:, :])

            ln1g_rep = ln1b_rep = ln2g_rep = ln2b_rep = None
            if flags["ln1_gb"]:
                ln1g_rep = singles.tile([P, DIM], f32, tag="ln1g")
                ln1b_rep = singles.tile([P, DIM], f32, tag="ln1b")
                nc.sync.dma_start(out=ln1g_rep, in_=bcast(opt["ln1g"][:]))
                nc.sync.dma_start(out=ln1b_rep, in_=bcast(opt["ln1b"][:]))
                nc.vector.tensor_scalar_mul(out=ln1b_rep, in0=ln1b_rep, scalar1=SH)
            if flags["ln2_gb"]:
                ln2g_rep = singles.tile([P, DIM], f32, tag="ln2g")
                ln2b_rep = singles.tile([P, DIM], f32, tag="ln2b")
                nc.sync.dma_start(out=ln2g_rep, in_=bcast(opt["ln2g"][:]))
                nc.sync.dma_start(out=ln2b_rep, in_=bcast(opt["ln2b"][:]))
                nc.vector.tensor_scalar_mul(out=ln2b_rep, in0=ln2b_rep, scalar1=SH)
            bqk_sb = bv_rep = bp_rep = bf1_sb = bf2_rep = None
            if flags["bqk"]:
                bqk_sb = singles.tile([P, 2 * KC], f32, tag="bqk")
                nc.sync.dma_start(
                    out=bqk_sb, in_=opt["bqk"][:].rearrange("(t p) -> p t", p=P)
                )
                nc.vector.tensor_scalar_mul(out=bqk_sb, in0=bqk_sb, scalar1=SQ)
            if flags["bv"]:
                bv_rep = singles.tile([P, DIM], f32, tag="bv")
                nc.sync.dma_start(out=bv_rep, in_=bcast(opt["bv"][:]))
            if flags["bp"]:
                bp_rep = singles.tile([P, DIM], f32, tag="bp")
                nc.sync.dma_start(out=bp_rep, in_=bcast(opt["bp"][:]))
            if flags["bf1"]:
                bf1_sb = singles.tile([P, KH], f32, tag="bf1")
                nc.sync.dma_start(
                    out=bf1_sb, in_=opt["bf1"][:].rearrange("(t p) -> p t", p=P)
                )
            if flags["bf2"]:
                bf2_rep = singles.tile([P, DIM], f32, tag="bf2")
                nc.sync.dma_start(out=bf2_rep, in_=bcast(opt["bf2"][:]))

            xq_r = xq_e.rearrange("(t p) c -> p t c", p=P)
            xk_r = xk_e.rearrange("(t p) c -> p t c", p=P)

            def ln_stats(x_ap, mv_ap):
                """bn_stats/aggr for one [128, 768] tile -> mv_ap [P, 2]."""
                stats = lnp.tile([P, 3, 6], f32, tag="ln_stats")
                xg = x_ap.rearrange("p (s d) -> p s d", s=3)
                for s in range(3):
                    nc.vector.bn_stats(out=stats[:, s, :], in_=xg[:, s, :])
                nc.vector.bn_aggr(out=mv_ap, in_=stats)

            def ln_rstd_batch(mv_all, rstd_all, n):
                """rstd*SH = exp(-0.5*ln((var+eps)/SH^2)) for n tiles at once.
                Uses the same Act table as softmax Exp (no table switch)."""
                lnv = lnp.tile([P, n], f32, tag="ln_lnv")
                nc.scalar.activation(
                    out=lnv, in_=mv_all[:, :, 1], func=ACT_F.Ln,
                    bias=eps_t, scale=1.0 / (SH * SH),
                )
                nc.scalar.activation(
                    out=rstd_all, in_=lnv, func=ACT_F.Exp, scale=-0.5,
                )

            def ln_apply(x_ap, out_ap, mv_ap, rstd_ap, g_rep, b_rep):
                """out = (x - mean) * (SH/std) [* g + b*SH]."""
                nc.vector.tensor_scalar(
                    out=out_ap, in0=x_ap,
                    scalar1=mv_ap, scalar2=rstd_ap,
                    op0=ALU.subtract, op1=ALU.mult,
                )
                if g_rep is not None:
                    nc.vector.tensor_mul(out=out_ap, in0=out_ap, in1=g_rep)
                if b_rep is not None:
                    nc.vector.tensor_add(out=out_ap, in0=out_ap, in1=b_rep)

            def transpose6(h_t, dstT, col):
                """six 128x128 transposes of h_t [128, 768] -> dstT[:, :, col]."""
                for kg in range(2):
                    pt = tps.tile([P, 4, P], bf16, tag="tp")
                    for j in range(3):
                        k = kg * 3 + j
                        nc.tensor.transpose(
                            pt[:, j, :], h_t[:, k * P : (k + 1) * P], ident
                        )
                    nc.vector.tensor_copy(
                        out=dstT[:, kg * 3 : kg * 3 + 3, col : col + P],
                        in_=pt[:, 0:3, :],
                    )

            for _rep in range(repeat):
                # --- x loads; own rows first (LN1 critical path) ---
                xt_own = big.tile([P, NT_O, DIM], f32, tag="xt_own")
                for t in range(NT_O):
                    nc.sync.dma_start(out=xt_own[:, t, :], in_=xq_r[:, t, :])
                xt_k = big.tile([P, KT, DIM], f32, tag="xt_k")
                for t in range(KT):
                    nc.sync.dma_start(out=xt_k[:, t, :], in_=xk_r[:, t, :])

                wqk_sb = big.tile([P, KC, 2 * DIM], f8, tag="wqk_wf2")
                for k in range(KC):
                    nc.sync.dma_start(
                        out=wqk_sb[:, k, :], in_=wqk_e[k * P : (k + 1) * P, :]
                    )
                wv_sb = big.tile([P, KC, DIM], f8, tag="wv_wp")
                for k in range(KC):
                    nc.sync.dma_start(
                        out=wv_sb[:, k, :], in_=wv_e[k * P : (k + 1) * P, :]
                    )

                # --- LN1 on own tiles -> hqT [128, KC, 512] fp8 (x SH) ---
                mv_q = lnp.tile([P, NT_O, 2], f32, tag="mv_q")
                for t in range(NT_O):
                    ln_stats(xt_own[:, t, :], mv_q[:, t, :])
                rstd_q = lnp.tile([P, NT_O], f32, tag="rstd_q")
                ln_rstd_batch(mv_q, rstd_q, NT_O)
                hqT = big.tile([P, KC, 512], f8, tag="hqT_h2T")
                for t in range(NT_O):
                    h_t = htmp.tile([P, DIM], bf16, tag="h")
                    ln_apply(xt_own[:, t, :], h_t, mv_q[:, t, 0:1],
                             rstd_q[:, t : t + 1], ln1g_rep, ln1b_rep)
                    transpose6(h_t, hqT, t * P)

                # --- LN1 on key tiles -> hkT [128, KC, KV] fp8 ---
                mv_k = lnp.tile([P, KT, 2], f32, tag="mv_k")
                for t in range(KT):
                    ln_stats(xt_k[:, t, :], mv_k[:, t, :])
                rstd_k = lnp.tile([P, KT], f32, tag="rstd_k")
                ln_rstd_batch(mv_k, rstd_k, KT)
                hkT = big.tile([P, KC, KV], f8, tag="hkT_oT")
                for t in range(KT):
                    h_t = htmp.tile([P, DIM], bf16, tag="h")
                    ln_apply(xt_k[:, t, :], h_t, mv_k[:, t, 0:1],
                             rstd_k[:, t : t + 1], ln1g_rep, ln1b_rep)
                    transpose6(h_t, hkT, t * P)

                # --- qT [ch, 512] fp8 (x SQ); kT [ch, KV] fp8 (x SQ) ---
                qT = big.tile([P, KC, 512], f8, tag="qT")
                kT = big.tile([P, KC, KV], f8, tag="kT")
                kv_chunks = []
                c0 = 0
                while c0 < KV:
                    c1 = min(c0 + 512, KV)
                    kv_chunks.append((c0, c1))
                    c0 = c1
                for mt in range(2 * KC):
                    is_q = mt < KC
                    for (t0, t1) in ([(0, 512)] if is_q else kv_chunks):
                        hsrc = hqT if is_q else hkT
                        ps_full = mmps.tile([P, 512], f32, tag="mm", name="mm")
                        ps = ps_full[:, : t1 - t0]
                        for kp in range(KC // 2):
                            nc.tensor.matmul(
                                ps,
                                lhsT=wqk_sb[:, 2 * kp : 2 * kp + 2,
                                            mt * P : (mt + 1) * P],
                                rhs=hsrc[:, 2 * kp : 2 * kp + 2, t0:t1],
                                start=(kp == 0),
                                stop=(kp == KC // 2 - 1),
                                perf_mode=DR,
                            )
                        if is_q:
                            dst = qT[:, mt, t0:t1]
                        else:
                            dst = kT[:, mt - KC, t0:t1]
                        if bqk_sb is not None:
                            nc.vector.tensor_scalar(
                                out=dst, in0=ps, scalar1=d_qk * SQ,
                                scalar2=bqk_sb[:, mt : mt + 1],
                                op0=ALU.mult, op1=ALU.add,
                            )
                        else:
                            nc.vector.tensor_scalar_mul(
                                out=dst, in0=ps, scalar1=d_qk * SQ
                            )

                # wf2 shares wqk's slot; emit its load now so the DMA runs
                # during attention, as soon as the last qk matmul releases wqk
                wf2_sb = big.tile([P, KH, DIM], bf16, tag="wqk_wf2")
                for k in range(KH):
                    nc.sync.dma_start(
                        out=wf2_sb[:, k, :], in_=wf2_e[k * P : (k + 1) * P, :]
                    )

                # --- v (x SV), masked: pad rows zeroed, per-head col 64 holds
                # m01*SV -- softmax numerator AND denominator exclude pads and
                # the SV cancels in the ratio ---
                v_aug = big.tile([P, KT, HEADS * 65], f8, tag="vaug_y")
                v_aug_h = v_aug.rearrange("p t (h c) -> p t h c", c=65)
                m01_bc = bass.AP(
                    tensor=m01_sb.tensor,
                    offset=m01_sb.offset,
                    ap=[m01_sb.ap[0], m01_sb.ap[1], [0, HEADS], [0, 1]],
                )
                nc.vector.tensor_scalar_mul(
                    out=v_aug_h[:, :, :, 64:65], in0=m01_bc, scalar1=SV
                )
                for t in range(KT):
                    for n0, n1 in ((0, 512), (512, 768)):
                        ps_full = mmps.tile([P, 512], f32, tag="mm", name="mm")
                        ps = ps_full[:, : n1 - n0]
                        for kp in range(KC // 2):
                            nc.tensor.matmul(
                                ps,
                                lhsT=hkT[:, 2 * kp : 2 * kp + 2,
                                         t * P : (t + 1) * P],
                                rhs=wv_sb[:, 2 * kp : 2 * kp + 2, n0:n1],
                                start=(kp == 0),
                                stop=(kp == KC // 2 - 1),
                                perf_mode=DR,
                            )
                        h0 = n0 // HD
                        h1 = n1 // HD
                        dst = v_aug_h[:, t, h0:h1, 0:HD]
                        src = ps.rearrange("p (h c) -> p h c", c=HD)
                        if bv_rep is not None:
                            tmpv = htmp.tile([P, n1 - n0], f32, tag="tmpv")
                            tr = tmpv.rearrange("p (h c) -> p h c", c=HD)
                            nc.vector.tensor_scalar_mul(out=tr, in0=src, scalar1=d_v)
                            nc.vector.tensor_add(
                                out=tr, in0=tr,
                                in1=bv_rep[:, n0:n1].rearrange("p (h c) -> p h c", c=HD),
                            )
                            nc.vector.tensor_scalar(
                                out=dst, in0=tr, scalar1=m01_sb[:, t : t + 1],
                                scalar2=SV, op0=ALU.mult, op1=ALU.mult,
                            )
                        else:
                            nc.vector.tensor_scalar(
                                out=dst, in0=src, scalar1=m01_sb[:, t : t + 1],
                                scalar2=d_v * SV, op0=ALU.mult, op1=ALU.mult,
                            )

                wf1_sb = big.tile([P, KC, HID], f8, tag="wf1")
                for k in range(KC):
                    for half in range(2):
                        nc.sync.dma_start(
                            out=wf1_sb[:, k, half * 1536 : (half + 1) * 1536],
                            in_=wf1_e[k * P : (k + 1) * P,
                                      half * 1536 : (half + 1) * 1536],
                        )

                # --- attention, head-pair at a time; the pair shares one
                # 2-bank psum so a single wide Exp covers both heads ---
                o_sb = big.tile([P, NT_O, DIM], bf16, tag="o_h2T")
                for hp in range(HEADS // 2):
                    pT = ppool.tile([P, KT, 2, 512], bf16, tag="pT")
                    for m in range(KT):
                        ps = sps.tile([P, 2, 512], f32, tag="s")
                        for sub in range(2):
                            base = sub * HD
                            nc.tensor.matmul(
                                ps[:, sub, :],
                                lhsT=kT[base : base + HD, hp, m * P : (m + 1) * P],
                                rhs=qT[base : base + HD, hp, :],
                                start=True,
                                stop=True,
                            )
                        nc.scalar.activation(
                            out=pT[:, m, :, :],
                            in_=ps,
                            func=ACT_F.Exp,
                            scale=float(HD) ** -0.5 / (SQ * SQ),
                        )
                    for sub in range(2):
                        h = 2 * hp + sub
                        for nt in range(NT_O):
                            po_full = mmps.tile([P, 512], f32, tag="mm", name="mm")
                            po = po_full[:, :65]
                            for m in range(KT):
                                nc.tensor.matmul(
                                    po,
                                    lhsT=pT[:, m, sub, nt * P : (nt + 1) * P],
                                    rhs=v_aug_h[:, m, h, :],
                                    start=(m == 0),
                                    stop=(m == KT - 1),
                                )
                            den = lnp.tile([P, 1], f32, tag="den")
                            nc.vector.tensor_scalar_mul(
                                out=den, in0=po[:, 64:65], scalar1=1.0 / SO
                            )
                            rcp = lnp.tile([P, 1], f32, tag="rcp")
                            nc.vector.reciprocal(out=rcp, in_=den)
                            nc.scalar.activation(
                                out=o_sb[:, nt, h * HD : (h + 1) * HD],
                                in_=po[:, 0:HD],
                                func=ACT_F.Copy,
                                scale=rcp,
                            )

                # --- oT (fp8, x SO) ---
                oT = big.tile([P, KC, 512], f8, tag="hkT_oT")
                for nt in range(NT_O):
                    for kg in range(2):
                        pt = tps.tile([P, 4, P], bf16, tag="tp")
                        for j in range(3):
                            k = kg * 3 + j
                            nc.tensor.transpose(
                                pt[:, j, :], o_sb[:, nt, k * P : (k + 1) * P], ident
                            )
                        nc.vector.tensor_copy(
                            out=oT[:, kg * 3 : kg * 3 + 3, nt * P : (nt + 1) * P],
                            in_=pt[:, 0:3, :],
                        )

                # --- proj + residual -> xmid f32 ---
                wp_sb = big.tile([P, KC, DIM], f8, tag="wv_wp")
                for k in range(KC):
                    nc.sync.dma_start(
                        out=wp_sb[:, k, :], in_=wp_e[k * P : (k + 1) * P, :]
                    )
                xmid = big.tile([P, NT_O, DIM], f32, tag="xmid")
                for nt in range(NT_O):
                    for n0, n1 in ((0, 512), (512, 768)):
                        ps_full = mmps.tile([P, 512], f32, tag="mm", name="mm")
                        ps = ps_full[:, : n1 - n0]
                        for kp in range(KC // 2):
                            nc.tensor.matmul(
                                ps,
                                lhsT=oT[:, 2 * kp : 2 * kp + 2,
                                        nt * P : (nt + 1) * P],
                                rhs=wp_sb[:, 2 * kp : 2 * kp + 2, n0:n1],
                                start=(kp == 0),
                                stop=(kp == KC // 2 - 1),
                                perf_mode=DR,
                            )
                        nc.vector.scalar_tensor_tensor(
                            out=xmid[:, nt, n0:n1], in0=ps, scalar=d_p,
                            in1=xt_own[:, nt, n0:n1],
                            op0=ALU.mult, op1=ALU.add,
                        )
                        if bp_rep is not None:
                            nc.vector.tensor_add(
                                out=xmid[:, nt, n0:n1],
                                in0=xmid[:, nt, n0:n1],
                                in1=bp_rep[:, n0:n1],
                            )

                # --- LN2 -> h2T (fp8, x SH) ---
                mv_2 = lnp.tile([P, NT_O, 2], f32, tag="mv_2")
                for nt in range(NT_O):
                    ln_stats(xmid[:, nt, :], mv_2[:, nt, :])
                rstd_2 = lnp.tile([P, NT_O], f32, tag="rstd_2")
                ln_rstd_batch(mv_2, rstd_2, NT_O)
                h2T = big.tile([P, KC, 512], f8, tag="hqT_h2T")
                for nt in range(NT_O):
                    h_t = htmp.tile([P, DIM], bf16, tag="h")
                    ln_apply(xmid[:, nt, :], h_t, mv_2[:, nt, 0:1],
                             rstd_2[:, nt : nt + 1], ln2g_rep, ln2b_rep)
                    transpose6(h_t, h2T, nt * P)

                # --- fc1^T + gelu -> g1T [128, KH, 512] bf16 ---
                g1T = big.tile([P, KH, 512], bf16, tag="kT_g1T")
                for mp2 in range(KH // 2):  # pairs of M-tiles share a 2-bank psum
                    ps2 = sps.tile([P, 2, 512], f32, tag="s")
                    for sub in range(2):
                        mt = 2 * mp2 + sub
                        for kp in range(KC // 2):
                            nc.tensor.matmul(
                                ps2[:, sub, :],
                                lhsT=wf1_sb[:, 2 * kp : 2 * kp + 2,
                                            mt * P : (mt + 1) * P],
                                rhs=h2T[:, 2 * kp : 2 * kp + 2, :],
                                start=(kp == 0),
                                stop=(kp == KC // 2 - 1),
                                perf_mode=DR,
                            )
                    if bf1_sb is not None:
                        for sub in range(2):
                            mt = 2 * mp2 + sub
                            nc.scalar.activation(
                                out=g1T[:, mt, :], in_=ps2[:, sub, :],
                                func=ACT_F.Gelu,
                                bias=bf1_sb[:, mt : mt + 1], scale=d_f1,
                            )
                    else:
                        nc.scalar.activation(
                            out=g1T[:, 2 * mp2 : 2 * mp2 + 2, :], in_=ps2,
                            func=ACT_F.Gelu, scale=d_f1,
                        )

                # --- fc2 (bf16) + residual -> y ---
                y_sb = big.tile([P, NT_O, DIM], f32, tag="vaug_y")
                y_r = y_e.rearrange("(t p) c -> p t c", p=P)
                for nt in range(NT_O):
                    for n0, n1 in ((0, 512), (512, 768)):
                        ps_full = mmps.tile([P, 512], f32, tag="mm", name="mm")
                        ps = ps_full[:, : n1 - n0]
                        for k in range(KH):
                            nc.tensor.matmul(
                                ps,
                                lhsT=g1T[:, k, nt * P : (nt + 1) * P],
                                rhs=wf2_sb[:, k, n0:n1],
                                start=(k == 0),
                                stop=(k == KH - 1),
                            )
                        nc.vector.tensor_add(
                            out=y_sb[:, nt, n0:n1], in0=ps, in1=xmid[:, nt, n0:n1]
                        )
                        if bf2_rep is not None:
                            nc.vector.tensor_add(
                                out=y_sb[:, nt, n0:n1],
                                in0=y_sb[:, nt, n0:n1],
                                in1=bf2_rep[:, n0:n1],
                            )
                        nc.sync.dma_start(
                            out=y_r[:, nt, n0:n1], in_=y_sb[:, nt, n0:n1]
                        )

    nc.finalize()
    return nc


def _nontriv(a, val):
    return not np.allclose(np.asarray(a), val, rtol=0, atol=0)


_last_flags = None


def _pow2_scale(w):
    """Largest power of two s with max|w|*s <= 120 (fp8e4 max is 240)."""
    m = float(np.abs(w).max())
    if m == 0.0:
        return 1.0
    return 2.0 ** int(np.floor(np.log2(120.0 / m)))


def _to_f8(w, s):
    return np.ascontiguousarray(w * s).astype(ml_dtypes.float8_e4m3)


def _prepare(x, attention_mask, ln1_g, ln1_b, ln2_g, ln2_b,
             w_qkv, b_qkv, w_proj, b_proj, w_fc1, b_fc1, w_fc2, b_fc2):
    x = np.ascontiguousarray(np.asarray(x, np.float32))
    attention_mask = np.asarray(attention_mask)
    B, N, C = x.shape
    H = N // 2  # 512

    kept = [np.flatnonzero(attention_mask[b] != 0) for b in range(B)]
    KT = max(1, -(-max(len(k) for k in kept) // P))

    flags = {
        "ln1_gb": _nontriv(ln1_g, 1.0) or _nontriv(ln1_b, 0.0),
        "ln2_gb": _nontriv(ln2_g, 1.0) or _nontriv(ln2_b, 0.0),
        "bqk": _nontriv(b_qkv[: 2 * DIM], 0.0),
        "bv": _nontriv(b_qkv[2 * DIM :], 0.0),
        "bp": _nontriv(b_proj, 0.0),
        "bf1": _nontriv(b_fc1, 0.0),
        "bf2": _nontriv(b_fc2, 0.0),
        "KT": KT,
    }
    KV = KT * P

    w_qkv = np.asarray(w_qkv, np.float32)
    wqk_f = np.ascontiguousarray(w_qkv[:, : 2 * DIM])
    wv_f = np.ascontiguousarray(w_qkv[:, 2 * DIM :])
    wp_f = np.asarray(w_proj, np.float32)
    wf1_f = np.asarray(w_fc1, np.float32)
    wf2_f = np.asarray(w_fc2, np.float32)

    scales = {
        "wqk": _pow2_scale(wqk_f),
        "wv": _pow2_scale(wv_f),
        "wp": _pow2_scale(wp_f),
        "wf1": _pow2_scale(wf1_f),
    }
    flags["scales"] = scales

    shared = {
        "wqk": _to_f8(wqk_f, scales["wqk"]),
        "wv": _to_f8(wv_f, scales["wv"]),
        "wp": _to_f8(wp_f, scales["wp"]),
        "wf1": _to_f8(wf1_f, scales["wf1"]),
        "wf2": wf2_f.astype(ml_dtypes.bfloat16),
    }
    if flags["ln1_gb"]:
        shared["ln1g"] = np.asarray(ln1_g, np.float32)
        shared["ln1b"] = np.asarray(ln1_b, np.float32)
    if flags["ln2_gb"]:
        shared["ln2g"] = np.asarray(ln2_g, np.float32)
        shared["ln2b"] = np.asarray(ln2_b, np.float32)
    if flags["bqk"]:
        shared["bqk"] = np.asarray(b_qkv[: 2 * DIM], np.float32)
    if flags["bv"]:
        shared["bv"] = np.asarray(b_qkv[2 * DIM :], np.float32)
    if flags["bp"]:
        shared["bp"] = np.asarray(b_proj, np.float32)
    if flags["bf1"]:
        shared["bf1"] = np.asarray(b_fc1, np.float32)
    if flags["bf2"]:
        shared["bf2"] = np.asarray(b_fc2, np.float32)

    in_maps = []
    for c in range(N_CORES):
        b, hf = divmod(c, 2)
        xq = np.ascontiguousarray(x[b, hf * H : (hf + 1) * H])
        idx = kept[b]
        xk = np.zeros((KV, C), np.float32)
        xk[: len(idx)] = x[b, idx]
        m01 = np.zeros(KV, np.float32)
        m01[: len(idx)] = 1.0
        m01 = np.ascontiguousarray(m01.reshape(KT, P).T)
        in_maps.append({"xq": xq, "xk": xk, "m01": m01, **shared})

    global _last_flags
    _last_flags = flags
    nc = _build(flags)
    return nc, in_maps, (B, N, C)


def kernel(**inputs):
    nc, in_maps, (B, N, C) = _prepare(**inputs)
    res = run_bass_kernel_spmd(nc, in_maps, list(range(N_CORES)))
    out = np.empty((B, N, C), np.float32)
    H = N // 2
    for c in range(N_CORES):
        b, hf = divmod(c, 2)
        out[b, hf * H : (hf + 1) * H] = res.results[c]["y"]
    return out
